# revision 9
# baseline (speedup 1.0000x reference)
"""DISCO (discrete-continuous) spherical conv encoder on 8 Trainium2 cores.

Strategy: output-latitude sharding (361 rows -> ~46/core), no collectives.
Host folds weight[o,c,k] x psi[k,h,l,d] x quad_w[lat_idx[h,l]] into per-h
matmul coefficients; device does per-latitude-group matmuls with PSUM
accumulation over the 9 longitude shifts (stride-2 rhs APs give the
PSCALE=2 decimation for free; a 4-col halo handles the longitude wrap).
"""
import os
import numpy as np
import ml_dtypes

B, CIN, COUT = 1, 16, 16
HIN, WIN = 721, 1440
HOUT, WOUT = 361, 720
KBAS, NL, ND = 9, 7, 9
NCORES = 8
HPC = 46          # valid output rows per core (last core: 39)
GRP = 8           # output rows per group
NG = 6            # groups per core (8*6=48 slots >= 46)
HBLK = NG * GRP   # 48
HALO = ND // 2    # 4
WROW = WIN + 2 * HALO  # 1448
NCHUNKS = ((0, 512), (512, WOUT - 512))  # psum-bank-aligned N split

_cache = {}
last_result = None


def _build_nc(RG, KT, kparts, dt_in):
    import concourse.bass as bass
    import concourse.bacc as bacc
    import concourse.mybir as mybir
    from concourse import tile

    nc = bacc.Bacc("TRN2", target_bir_lowering=False, debug=False,
                   num_devices=NCORES)
    xr = nc.declare_dram_parameter("xr", [CIN, NG * RG, WIN], dt_in,
                                   isOutput=False)
    w2 = nc.declare_dram_parameter("w2", [NG, 128, ND * KT * 128], dt_in,
                                   isOutput=False)
    y = nc.declare_dram_parameter("y", [COUT, HBLK, WOUT], mybir.dt.float32,
                                  isOutput=True)
    xr_t = xr.ap().transpose([1, 0, 2])  # [row, c, w]
    y_t = y.ap().transpose([1, 0, 2])    # [h, o, w]

    with tile.TileContext(nc) as tc:
        with (
            tc.tile_pool(name="rbp", bufs=2) as rbp,
            tc.tile_pool(name="w2p", bufs=2) as w2p,
            tc.tile_pool(name="psp", bufs=2, space="PSUM") as psp,
            tc.tile_pool(name="outp", bufs=2) as outp,
        ):
            for g in range(NG):
                w2t = w2p.tile([128, ND * KT * 128], dt_in, tag="w2")
                nc.sync.dma_start(out=w2t[:, :], in_=w2.ap()[g])
                rbs = []
                for kt in range(KT):
                    nrows = (kparts[kt] + 15) // 16  # rows in this k-tile
                    rb = rbp.tile([128, WROW], dt_in, tag=f"rb{kt}")
                    np_ = nrows * CIN
                    r0 = g * RG + 8 * kt  # first xr row of this tile
                    # body + wrap halos; partitions = (row, c), src 3D
                    for dst_c0, src_c0, ncol in (
                        (HALO, 0, WIN),
                        (0, WIN - HALO, HALO),
                        (HALO + WIN, 0, HALO),
                    ):
                        nc.sync.dma_start(
                            out=rb[0:np_, dst_c0:dst_c0 + ncol],
                            in_=xr_t[r0:r0 + nrows, :, src_c0:src_c0 + ncol])
                    rbs.append(rb)
                pss = [psp.tile([128, nw], mybir.dt.float32, tag=f"ps{ci}",
                                name=f"ps{ci}_{g}")
                       for ci, (w0, nw) in enumerate(NCHUNKS)]
                for d in range(ND):
                    for kt in range(KT):
                        kp = kparts[kt]
                        lhsT = w2t[0:kp, (d * KT + kt) * 128:
                                   (d * KT + kt) * 128 + 128]
                        first = d == 0 and kt == 0
                        last = d == ND - 1 and kt == KT - 1
                        for ci, (w0, nw) in enumerate(NCHUNKS):
                            c0 = d + 2 * w0
                            nc.tensor.matmul(
                                pss[ci][:, :], lhsT,
                                rbs[kt][0:kp, c0:c0 + 2 * nw:2],
                                start=first, stop=last)
                stage = outp.tile([128, WOUT], mybir.dt.float32, tag="stage",
                                  name=f"stage_{g}")
                for ci, (w0, nw) in enumerate(NCHUNKS):
                    nc.vector.tensor_copy(out=stage[:, w0:w0 + nw],
                                          in_=pss[ci][:, :])
                nc.sync.dma_start(
                    out=y_t[g * GRP:(g + 1) * GRP, :, :], in_=stage[:, :])
    nc.compile()
    return nc


def _prepare(x, psi, weight, quad_w, lat_idx):
    x = np.asarray(x)
    psi = np.asarray(psi)
    weight = np.asarray(weight)
    quad_w = np.asarray(quad_w)
    lat = np.clip(np.asarray(lat_idx).astype(np.int64), 0, HIN - 1)

    use_f32 = os.environ.get("KERNEL_DTYPE", "bf16") == "f32"
    np_dt = np.float32 if use_f32 else ml_dtypes.bfloat16

    # ---- host fold: W2[h, l, d, c, o]
    psi_q = psi.astype(np.float64) * \
        quad_w.astype(np.float64)[lat][None, :, :, None]
    W2 = np.einsum("ock,khld->hldco", weight.astype(np.float64),
                   psi_q).astype(np.float32)

    # ---- per-core plan (generic in lat_idx; structured input -> RG=21)
    plans = []
    RG = 1
    for i in range(NCORES):
        h0 = i * HPC
        h1 = min(HOUT, h0 + HPC)
        groups = []
        for g in range(NG):
            hs = h0 + g * GRP
            he = min(h1, hs + GRP)
            rows_g = np.unique(lat[hs:he]) if hs < he else np.zeros(
                1, np.int64)
            RG = max(RG, len(rows_g))
            groups.append((hs, he, rows_g))
        plans.append((h0, h1, groups))
    KTOT = RG * CIN
    KT = (KTOT + 127) // 128
    kparts = [min(128, KTOT - kt * 128) for kt in range(KT)]

    # ---- per-core host arrays
    in_maps = []
    x0 = x[0]  # [CIN, HIN, WIN]
    for h0, h1, groups in plans:
        xr = np.zeros((CIN, NG * RG, WIN), np.float32)
        w2h = np.zeros((NG, 128, ND, KT, 128), np.float32)
        for g, (hs, he, rows_g) in enumerate(groups):
            nr = len(rows_g)
            xr[:, g * RG:g * RG + nr, :] = x0[:, rows_g, :]
            for hsub in range(he - hs):
                h = hs + hsub
                js = np.searchsorted(rows_g, lat[h])  # [NL]
                for l in range(NL):
                    j = js[l]
                    q = j * 16
                    # [ND, C, O] -> [C, ND, O]
                    blk = W2[h, l].transpose(1, 0, 2)
                    w2h[g, q % 128:q % 128 + 16, :, q // 128,
                        hsub * 16:hsub * 16 + 16] += blk
        in_maps.append({
            "xr": xr.astype(np_dt),
            "w2": np.ascontiguousarray(
                w2h.reshape(NG, 128, ND * KT * 128)).astype(np_dt),
        })
    return in_maps, plans, RG, KT, kparts, use_f32


def kernel(x, psi, weight, quad_w, lat_idx):
    global last_result
    x = np.asarray(x)
    in_maps, plans, RG, KT, kparts, use_f32 = _prepare(
        x, psi, weight, quad_w, lat_idx)

    # ---- build & run
    from concourse.bass_utils import run_bass_kernel_spmd
    import concourse.mybir as mybir
    dt_in = mybir.dt.float32 if use_f32 else mybir.dt.bfloat16
    key = (RG, KT, tuple(kparts), str(dt_in))
    if key not in _cache:
        _cache[key] = _build_nc(RG, KT, kparts, dt_in)
    nc = _cache[key]

    trace = os.environ.get("KERNEL_TRACE") == "1"
    try:
        res = run_bass_kernel_spmd(nc, in_maps, list(range(NCORES)),
                                   trace=trace)
    except ModuleNotFoundError:
        if not trace:
            raise
        res = run_bass_kernel_spmd(nc, in_maps, list(range(NCORES)),
                                   trace=False)
    last_result = res

    out = np.empty((B, COUT, HOUT, WOUT), np.float32)
    for i, (h0, h1, _) in enumerate(plans):
        out[0, :, h0:h1, :] = res.results[i]["y"][:, :h1 - h0, :]
    return out.astype(x.dtype)


# revision 15
# speedup vs baseline: 1.3148x; 1.3148x over previous
"""DISCO (discrete-continuous) spherical conv encoder on 8 Trainium2 cores.

Strategy: output-latitude sharding (361 rows -> ~46/core), no collectives.
Host folds weight[o,c,k] x psi[k,h,l,d] x quad_w[lat_idx[h,l]] into per-h
matmul coefficients; device does per-latitude-group matmuls with PSUM
accumulation over the 9 longitude shifts (stride-2 rhs APs give the
PSCALE=2 decimation for free; a 4-col halo handles the longitude wrap).
"""
import os
import numpy as np
import ml_dtypes

B, CIN, COUT = 1, 16, 16
HIN, WIN = 721, 1440
HOUT, WOUT = 361, 720
KBAS, NL, ND = 9, 7, 9
NCORES = 8
HPC = 46          # valid output rows per core (last core: 39)
GRP = 8           # output rows per group
NG = 6            # groups per core (8*6=48 slots >= 46)
HBLK = NG * GRP   # 48
HALO = ND // 2    # 4
WROW = WIN + 2 * HALO  # 1448
NCHUNKS = ((0, 512), (512, WOUT - 512))  # psum-bank-aligned N split

# ---- v2 sliding-window scheme constants
HPC2 = 48         # output rows per core (48*8=384 >= 361); 48%8==0 keeps the
                  # block->slot mapping identical on every core (SPMD)
NT = 14           # aligned 8-input-row blocks per core (incl leading dummy)
NPS = 4           # rotating PSUM accumulators (4 x 2 banks = all 8 banks)

_cache = {}
last_result = None


def _build_nc(RG, KT, kparts, dt_in):
    import concourse.bass as bass
    import concourse.bacc as bacc
    import concourse.mybir as mybir
    from concourse import tile

    nc = bacc.Bacc("TRN2", target_bir_lowering=False, debug=False,
                   num_devices=NCORES)
    xr = nc.declare_dram_parameter("xr", [CIN, NG * RG, WIN], dt_in,
                                   isOutput=False)
    w2 = nc.declare_dram_parameter("w2", [NG, 128, ND * KT * 128], dt_in,
                                   isOutput=False)
    y = nc.declare_dram_parameter("y", [COUT, HBLK, WOUT], mybir.dt.float32,
                                  isOutput=True)
    xr_t = xr.ap().transpose([1, 0, 2])  # [row, c, w]
    y_t = y.ap().transpose([1, 0, 2])    # [h, o, w]

    with tile.TileContext(nc) as tc:
        with (
            tc.tile_pool(name="rbp", bufs=2) as rbp,
            tc.tile_pool(name="w2p", bufs=2) as w2p,
            tc.tile_pool(name="psp", bufs=2, space="PSUM") as psp,
            tc.tile_pool(name="outp", bufs=2) as outp,
        ):
            for g in range(NG):
                w2t = w2p.tile([128, ND * KT * 128], dt_in, tag="w2")
                nc.sync.dma_start(out=w2t[:, :], in_=w2.ap()[g])
                rbs = []
                for kt in range(KT):
                    nrows = (kparts[kt] + 15) // 16  # rows in this k-tile
                    rb = rbp.tile([128, WROW], dt_in, tag=f"rb{kt}")
                    np_ = nrows * CIN
                    r0 = g * RG + 8 * kt  # first xr row of this tile
                    # body + wrap halos; partitions = (row, c), src 3D
                    for dst_c0, src_c0, ncol in (
                        (HALO, 0, WIN),
                        (0, WIN - HALO, HALO),
                        (HALO + WIN, 0, HALO),
                    ):
                        nc.sync.dma_start(
                            out=rb[0:np_, dst_c0:dst_c0 + ncol],
                            in_=xr_t[r0:r0 + nrows, :, src_c0:src_c0 + ncol])
                    rbs.append(rb)
                pss = [psp.tile([128, nw], mybir.dt.float32, tag=f"ps{ci}",
                                name=f"ps{ci}_{g}")
                       for ci, (w0, nw) in enumerate(NCHUNKS)]
                for d in range(ND):
                    for kt in range(KT):
                        kp = kparts[kt]
                        lhsT = w2t[0:kp, (d * KT + kt) * 128:
                                   (d * KT + kt) * 128 + 128]
                        first = d == 0 and kt == 0
                        last = d == ND - 1 and kt == KT - 1
                        for ci, (w0, nw) in enumerate(NCHUNKS):
                            c0 = d + 2 * w0
                            nc.tensor.matmul(
                                pss[ci][:, :], lhsT,
                                rbs[kt][0:kp, c0:c0 + 2 * nw:2],
                                start=first, stop=last)
                stage = outp.tile([128, WOUT], mybir.dt.float32, tag="stage",
                                  name=f"stage_{g}")
                for ci, (w0, nw) in enumerate(NCHUNKS):
                    nc.vector.tensor_copy(out=stage[:, w0:w0 + nw],
                                          in_=pss[ci][:, :])
                nc.sync.dma_start(
                    out=y_t[g * GRP:(g + 1) * GRP, :, :], in_=stage[:, :])
    nc.compile()
    return nc


def _build_nc_v2(dt_in):
    """Sliding-window scheme: one K=128 matmul tile per aligned 8-input-row
    block x 9 lon shifts, accumulating into a rotating set of 4 PSUM tiles
    with output slot = (local output row) mod 8.  After block tau, output
    rows 4*tau-5 .. 4*tau-2 (local) are complete: rows spanning two blocks
    are summed from two PSUM tiles on the Vector engine, single-block rows
    are copied, and the result is DMA'd to y rows [4*tau, 4*tau+4)."""
    import concourse.bacc as bacc
    import concourse.mybir as mybir
    from concourse import tile

    nc = bacc.Bacc("TRN2", target_bir_lowering=False, debug=False,
                   num_devices=NCORES)
    xv = nc.declare_dram_parameter("xv", [NT, 128, WROW], dt_in,
                                   isOutput=False)
    w2v = nc.declare_dram_parameter("w2v", [NT, 128, ND * 128], dt_in,
                                    isOutput=False)
    y = nc.declare_dram_parameter("y", [4 * NT, COUT, WOUT],
                                  mybir.dt.float32, isOutput=True)

    with tile.TileContext(nc) as tc:
        with (
            tc.tile_pool(name="xbp", bufs=3) as xbp,
            tc.tile_pool(name="wbp", bufs=3) as wbp,
            tc.tile_pool(name="psp", bufs=1, space="PSUM") as psp,
            tc.tile_pool(name="stp", bufs=3) as stp,
        ):
            P = [psp.tile([128, WOUT], mybir.dt.float32, tag=f"P{j}",
                          name=f"P{j}") for j in range(NPS)]
            for tau in range(NT):
                xb = xbp.tile([128, WROW], dt_in, tag="xb", name=f"xb{tau}")
                nc.sync.dma_start(out=xb[:, :], in_=xv.ap()[tau])
                wb = wbp.tile([128, ND * 128], dt_in, tag="wb",
                              name=f"wb{tau}")
                nc.sync.dma_start(out=wb[:, :], in_=w2v.ap()[tau])
                ps = P[tau % NPS]
                for d in range(ND):
                    lhsT = wb[:, d * 128:(d + 1) * 128]
                    for w0, nw in NCHUNKS:
                        c0 = d + 2 * w0
                        nc.tensor.matmul(ps[:, w0:w0 + nw], lhsT,
                                         xb[:, c0:c0 + 2 * nw:2],
                                         start=(d == 0), stop=(d == ND - 1))
                # flush the 4 output rows finished by this block; with
                # slot = (h_local+1)%8 the flush set is one 64-partition
                # range at base 64 (even tau) / 0 (odd tau).  The j==3
                # (single-block) row's prev-tile contribution is exactly 0
                # (its weights there are zero), so one add covers all 4.
                st = stp.tile([128, WOUT], mybir.dt.float32, tag="st",
                              name=f"st{tau}")
                prev = P[(tau - 1) % NPS]
                p0 = 64 if tau % 2 == 0 else 0
                if tau == 0:
                    # dummy flush: no valid rows; keep the program uniform
                    nc.vector.tensor_copy(out=st[p0:p0 + 64, :],
                                          in_=ps[p0:p0 + 64, :])
                else:
                    # DVE reads at most one PSUM operand: stage prev first
                    cp = stp.tile([128, WOUT], mybir.dt.float32, tag="cp",
                                  name=f"cp{tau}")
                    nc.vector.tensor_copy(out=cp[p0:p0 + 64, :],
                                          in_=prev[p0:p0 + 64, :])
                    nc.vector.tensor_add(out=st[p0:p0 + 64, :],
                                         in0=cp[p0:p0 + 64, :],
                                         in1=ps[p0:p0 + 64, :])
                nc.sync.dma_start(out=y.ap()[4 * tau:4 * tau + 4],
                                  in_=st[p0:p0 + 64, :])
    nc.compile()
    return nc


def _v2_valid(lat):
    """v2 requires each output row's input rows to sit in the 1-2 aligned
    8-row blocks implied by its flush position (true for the structured
    equiangular lat_idx; arbitrary indices fall back to the group scheme)."""
    for i in range(NCORES):
        h0 = i * HPC2
        for h in range(h0, min(HOUT, h0 + HPC2)):
            hl = h - h0
            tf = (hl + 5) // 4
            j = (hl + 5) % 4
            if tf >= NT:
                return False
            gbf = 12 * i - 1 + tf
            blocks = set(int(r) // 8 for r in lat[h])
            allowed = {gbf} if j == 3 else {gbf - 1, gbf}
            if not blocks <= allowed:
                return False
    return True


def _prepare_v2(x, psi, weight, quad_w, lat):
    use_f32 = os.environ.get("KERNEL_DTYPE", "bf16") == "f32"
    np_dt = np.float32 if use_f32 else ml_dtypes.bfloat16

    psi_q = psi.astype(np.float64) * \
        quad_w.astype(np.float64)[lat][None, :, :, None]
    W2 = np.einsum("ock,khld->hldco", weight.astype(np.float64),
                   psi_q).astype(np.float32)

    in_maps = []
    x0 = x[0]
    for i in range(NCORES):
        h0 = i * HPC2
        h1 = min(HOUT, h0 + HPC2)
        xv = np.zeros((NT, 128, WROW), np.float32)
        w2v = np.zeros((NT, 128, ND, 128), np.float32)
        for tau in range(NT):
            gb = 12 * i - 1 + tau
            for rl in range(8):
                rho = 8 * gb + rl
                if 0 <= rho < HIN:
                    row = x0[:, rho, :]  # [CIN, WIN]
                    xv[tau, rl * 16:rl * 16 + 16, HALO:HALO + WIN] = row
                    xv[tau, rl * 16:rl * 16 + 16, :HALO] = row[:, WIN - HALO:]
                    xv[tau, rl * 16:rl * 16 + 16, HALO + WIN:] = row[:, :HALO]
        for h in range(h0, h1):
            hl = h - h0
            ms = ((hl + 1) % 8) * 16
            for l in range(NL):
                rho = int(lat[h, l])
                tau = rho // 8 - (12 * i - 1)
                ps = (rho % 8) * 16
                # [ND, C, O] -> [C, ND, O]
                w2v[tau, ps:ps + 16, :, ms:ms + 16] += \
                    W2[h, l].transpose(1, 0, 2)
        in_maps.append({
            "xv": xv.astype(np_dt),
            "w2v": np.ascontiguousarray(
                w2v.reshape(NT, 128, ND * 128)).astype(np_dt),
        })
    return in_maps, use_f32


def _prepare(x, psi, weight, quad_w, lat_idx):
    x = np.asarray(x)
    psi = np.asarray(psi)
    weight = np.asarray(weight)
    quad_w = np.asarray(quad_w)
    lat = np.clip(np.asarray(lat_idx).astype(np.int64), 0, HIN - 1)

    use_f32 = os.environ.get("KERNEL_DTYPE", "bf16") == "f32"
    np_dt = np.float32 if use_f32 else ml_dtypes.bfloat16

    # ---- host fold: W2[h, l, d, c, o]
    psi_q = psi.astype(np.float64) * \
        quad_w.astype(np.float64)[lat][None, :, :, None]
    W2 = np.einsum("ock,khld->hldco", weight.astype(np.float64),
                   psi_q).astype(np.float32)

    # ---- per-core plan (generic in lat_idx; structured input -> RG=21)
    plans = []
    RG = 1
    for i in range(NCORES):
        h0 = i * HPC
        h1 = min(HOUT, h0 + HPC)
        groups = []
        for g in range(NG):
            hs = h0 + g * GRP
            he = min(h1, hs + GRP)
            rows_g = np.unique(lat[hs:he]) if hs < he else np.zeros(
                1, np.int64)
            RG = max(RG, len(rows_g))
            groups.append((hs, he, rows_g))
        plans.append((h0, h1, groups))
    KTOT = RG * CIN
    KT = (KTOT + 127) // 128
    kparts = [min(128, KTOT - kt * 128) for kt in range(KT)]

    # ---- per-core host arrays
    in_maps = []
    x0 = x[0]  # [CIN, HIN, WIN]
    for h0, h1, groups in plans:
        xr = np.zeros((CIN, NG * RG, WIN), np.float32)
        w2h = np.zeros((NG, 128, ND, KT, 128), np.float32)
        for g, (hs, he, rows_g) in enumerate(groups):
            nr = len(rows_g)
            xr[:, g * RG:g * RG + nr, :] = x0[:, rows_g, :]
            for hsub in range(he - hs):
                h = hs + hsub
                js = np.searchsorted(rows_g, lat[h])  # [NL]
                for l in range(NL):
                    j = js[l]
                    q = j * 16
                    # [ND, C, O] -> [C, ND, O]
                    blk = W2[h, l].transpose(1, 0, 2)
                    w2h[g, q % 128:q % 128 + 16, :, q // 128,
                        hsub * 16:hsub * 16 + 16] += blk
        in_maps.append({
            "xr": xr.astype(np_dt),
            "w2": np.ascontiguousarray(
                w2h.reshape(NG, 128, ND * KT * 128)).astype(np_dt),
        })
    return in_maps, plans, RG, KT, kparts, use_f32


def _run(nc, in_maps):
    from concourse.bass_utils import run_bass_kernel_spmd
    trace = os.environ.get("KERNEL_TRACE") == "1"
    try:
        return run_bass_kernel_spmd(nc, in_maps, list(range(NCORES)),
                                    trace=trace)
    except ModuleNotFoundError:
        if not trace:
            raise
        return run_bass_kernel_spmd(nc, in_maps, list(range(NCORES)),
                                    trace=False)


def kernel(x, psi, weight, quad_w, lat_idx):
    global last_result
    import concourse.mybir as mybir
    x = np.asarray(x)
    psi = np.asarray(psi)
    weight = np.asarray(weight)
    quad_w = np.asarray(quad_w)
    lat = np.clip(np.asarray(lat_idx).astype(np.int64), 0, HIN - 1)

    scheme = os.environ.get("KERNEL_SCHEME", "auto")
    use_v2 = scheme != "v1" and (scheme == "v2" or _v2_valid(lat))

    if use_v2:
        in_maps, use_f32 = _prepare_v2(x, psi, weight, quad_w, lat)
        dt_in = mybir.dt.float32 if use_f32 else mybir.dt.bfloat16
        key = ("v2", str(dt_in))
        if key not in _cache:
            _cache[key] = _build_nc_v2(dt_in)
        res = _run(_cache[key], in_maps)
        last_result = res
        out = np.empty((B, COUT, HOUT, WOUT), np.float32)
        for i in range(NCORES):
            h0 = i * HPC2
            h1 = min(HOUT, h0 + HPC2)
            out[0, :, h0:h1, :] = \
                res.results[i]["y"][5:5 + h1 - h0].transpose(1, 0, 2)
        return out.astype(x.dtype)

    in_maps, plans, RG, KT, kparts, use_f32 = _prepare(
        x, psi, weight, quad_w, lat_idx)
    dt_in = mybir.dt.float32 if use_f32 else mybir.dt.bfloat16
    key = (RG, KT, tuple(kparts), str(dt_in))
    if key not in _cache:
        _cache[key] = _build_nc(RG, KT, kparts, dt_in)
    res = _run(_cache[key], in_maps)
    last_result = res

    out = np.empty((B, COUT, HOUT, WOUT), np.float32)
    for i, (h0, h1, _) in enumerate(plans):
        out[0, :, h0:h1, :] = res.results[i]["y"][:, :h1 - h0, :]
    return out.astype(x.dtype)
